# revision 1
# baseline (speedup 1.0000x reference)
"""Liquid State Machine on 8 Trainium2 NeuronCores.

Strategy: shard the reservoir (R=2000, padded to 2048) across 8 cores
(256 rows each); replicate the batch (B=32). Each timestep, every core
computes input+recurrent currents for its 256 neurons with a
weights-stationary fp32 matmul (lhsT = W_res_loc.T tiles, rhs = full
spike vector [2048, 32]), updates the adaptive-LIF state on the vector
engine, and the 8 cores exchange their spike blocks with an AllGather.
State layout is [128 partitions, 2*32] (neurons on partitions, batch on
the free dim), so the AllGather output concatenates rank blocks directly
into the next step's matmul rhs. Readout features (final/mean/rate/
weighted membrane stats) accumulate on-device; the tiny [32,8000]@[8000,10]
readout runs on host.
"""
import os
from contextlib import ExitStack

import numpy as np

import concourse.bass as bass
import concourse.bacc as bacc
import concourse.tile as tile
from concourse import mybir
from concourse.bass_utils import run_bass_kernel_spmd

N_CORES = 8
B = 32
T = 200
NI = 256
R = 2000
RP = 2048          # padded reservoir
RLOC = RP // N_CORES   # 256 rows per core
TAU_INV = np.float32(1.0 / 20.0)
F32 = mybir.dt.float32
F32R = mybir.dt.float32r

_cached = {}


def _build_program(n_steps=T, exchange="cc"):
    key = ("nc", n_steps, exchange)
    if key in _cached:
        return _cached[key]
    nc = bacc.Bacc("TRN2", target_bir_lowering=False, debug=False,
                   num_devices=N_CORES)

    wres_d = nc.dram_tensor("wres", [128, 16, 256], F32, kind="ExternalInput")
    iin_d = nc.dram_tensor("iin", [128, 2, T, 32], F32, kind="ExternalInput")
    feats_d = nc.dram_tensor("feats", [4, 128, 64], F32, kind="ExternalOutput")

    with tile.TileContext(nc) as tc:
        with ExitStack() as ctx:
            sb = ctx.enter_context(tc.tile_pool(name="sb", bufs=1))
            ps_pool = ctx.enter_context(
                tc.tile_pool(name="ps", bufs=2, space="PSUM"))
            dram = ctx.enter_context(
                tc.tile_pool(name="dram", bufs=1, space="DRAM"))

            wres = sb.tile([128, 16, 256], F32)
            nc.sync.dma_start(out=wres[:], in_=wres_d[:])
            iin = sb.tile([128, 2, T, 32], F32)
            nc.sync.dma_start(out=iin[:], in_=iin_d[:])

            # ping-pong full-spike buffers, viewed as [128, 16*32]:
            # K-tile k lives at free columns [32k, 32k+32)
            sfull0 = sb.tile([128, 8, 64], F32)
            sfull1 = sb.tile([128, 8, 64], F32)
            sfull = [sfull0, sfull1]
            nc.vector.memset(sfull0[:], 0.0)

            v = sb.tile([128, 64], F32)
            A = sb.tile([128, 64], F32)      # adaptive threshold = 1 + a
            sv = sb.tile([128, 64], F32)
            ss = sb.tile([128, 64], F32)
            swv = sb.tile([128, 64], F32)
            zeros = sb.tile([128, 64], F32)
            s_loc = sb.tile([128, 64], F32)
            tmp = sb.tile([128, 64], F32)
            thr = sb.tile([128, 64], F32)
            nc.vector.memset(v[:], 0.0)
            nc.vector.memset(A[:], 1.0)
            nc.vector.memset(sv[:], 0.0)
            nc.vector.memset(ss[:], 0.0)
            nc.vector.memset(swv[:], 0.0)
            nc.vector.memset(zeros[:], 0.0)

            dw = np.exp(-np.arange(T, dtype=np.float64) / 10.0).astype(np.float32)

            for t in range(n_steps):
                cur = sfull[t % 2]       # holds spikes(t-1)
                nxt = sfull[(t + 1) % 2]
                cur_flat = cur.rearrange("p r x -> p (r x)")

                ps = ps_pool.tile([128, 64], F32)
                for m in range(2):
                    for k in range(16):
                        nc.tensor.matmul(
                            ps[:, 32 * m:32 * m + 32],
                            wres[:, k, 128 * m:128 * m + 128],
                            cur_flat[:, 32 * k:32 * k + 32],
                            start=(k == 0),
                            stop=(k == 15),
                        )

                # pre-threshold work that overlaps the matmuls:
                # v_pre = 0.95 v + iin_t;  thr = A - v_pre
                # spike test (v_pre + ps >= A) becomes ps >= thr, so the
                # only post-matmul ops on the exchange path are one is_ge
                nc.vector.tensor_scalar_mul(v[:], v[:], 0.95)
                nc.vector.tensor_add(v[:], v[:], iin[:, :, t, :])
                nc.vector.tensor_sub(thr[:], A[:], v[:])
                nc.vector.tensor_tensor(s_loc[:], ps[:], thr[:],
                                        mybir.AluOpType.is_ge)
                # off the critical path: full v update + reset
                nc.vector.tensor_add(v[:], v[:], ps[:])
                nc.vector.tensor_mul(tmp[:], v[:], s_loc[:])
                nc.vector.tensor_sub(v[:], v[:], tmp[:])
                # threshold adaptation: A = 0.99 A + 0.01 + 0.1 s
                nc.vector.tensor_scalar(A[:], A[:], 0.99, 0.01,
                                        mybir.AluOpType.mult, mybir.AluOpType.add)
                nc.vector.tensor_scalar_mul(tmp[:], s_loc[:], 0.1)
                nc.vector.tensor_add(A[:], A[:], tmp[:])
                # feature accumulators
                nc.gpsimd.tensor_add(sv[:], sv[:], v[:])
                nc.gpsimd.tensor_add(ss[:], ss[:], s_loc[:])
                nc.vector.tensor_scalar_mul(tmp[:], v[:], float(dw[t]))
                nc.vector.tensor_add(swv[:], swv[:], tmp[:])

                # exchange spike blocks (per-step collective buffers: Shared
                # DRAM wants a single writer per tensor)
                if exchange == "cc":
                    cc_in = dram.tile([128, 64], F32, name=f"cc_in_{t}")
                    cc_out = dram.tile([N_CORES, 128, 64], F32,
                                       addr_space="Shared", name=f"cc_out_{t}")
                    nc.sync.dma_start(out=cc_in[:], in_=s_loc[:])
                    nc.gpsimd.collective_compute(
                        "AllGather",
                        mybir.AluOpType.bypass,
                        replica_groups=[list(range(N_CORES))],
                        ins=[cc_in.opt()],
                        outs=[cc_out.opt()],
                    )
                    half = cc_out.rearrange("r p x -> p r x")
                    nc.sync.dma_start(out=nxt[:, 0:4, :], in_=half[:, 0:4, :])
                    nc.scalar.dma_start(out=nxt[:, 4:8, :], in_=half[:, 4:8, :])
                elif exchange == "local":
                    # timing-only variant: fake the exchange with local copies
                    # (keeps the spikes->next-matmul dependency, wrong data)
                    for rr in range(N_CORES):
                        nc.vector.tensor_copy(nxt[:, rr, :], s_loc[:])
                elif exchange == "none":
                    pass

            nc.sync.dma_start(out=feats_d[0], in_=v[:])
            nc.sync.dma_start(out=feats_d[1], in_=sv[:])
            nc.sync.dma_start(out=feats_d[2], in_=ss[:])
            nc.sync.dma_start(out=feats_d[3], in_=swv[:])

    nc.compile()
    _cached[key] = nc
    return nc


def kernel(x_input, W_input, W_reservoir, W_readout, b_readout,
           _trace=False, _trace_kwargs=None, _n_steps=T, _timing=None):
    x = np.ascontiguousarray(x_input, dtype=np.float32)
    W_in = np.asarray(W_input, np.float32)
    W_res = np.asarray(W_reservoir, np.float32)
    W_ro = np.asarray(W_readout, np.float32)
    b_ro = np.asarray(b_readout, np.float32)

    # pre-scaled (x 1/tau), padded weights
    Wp = np.zeros((RP, RP), np.float32)
    Wp[:R, :R] = W_res
    Wp *= TAU_INV
    Wip = np.zeros((RP, NI), np.float32)
    Wip[:R] = W_in

    # input currents for all steps: [B*T, RP] (row = b*T + t)
    xw = (x.reshape(B * T, NI) @ Wip.T).astype(np.float32) * TAU_INV

    in_maps = []
    for c in range(N_CORES):
        wl = Wp[RLOC * c:RLOC * (c + 1), :]            # [256, 2048]
        # lhsT tiles: [128(kpart), 16(ktile), 256(m)]
        wres_c = np.ascontiguousarray(
            wl.T.reshape(16, 128, 256).transpose(1, 0, 2))
        ic = xw.reshape(B, T, RP)[:, :, RLOC * c:RLOC * (c + 1)]  # [B,T,256]
        iin_c = np.ascontiguousarray(
            ic.reshape(B, T, 2, 128).transpose(3, 2, 1, 0))  # [128,2,T,32]
        in_maps.append({"wres": wres_c, "iin": iin_c})

    nc = _build_program(_n_steps)
    import time as _time
    _t0 = _time.time()
    res = run_bass_kernel_spmd(
        nc, in_maps, list(range(N_CORES)),
        trace=_trace, **(_trace_kwargs or {}))
    if _timing is not None:
        _timing.append(_time.time() - _t0)
    if _trace:
        _cached["last_result"] = res

    # assemble features: [4, 2048, 32]
    full = np.zeros((4, RP, B), np.float32)
    for c in range(N_CORES):
        f = res.results[c]["feats"]  # [4, 128, 64]
        blk = f.reshape(4, 128, 2, 32).transpose(0, 2, 1, 3).reshape(4, 256, 32)
        full[:, RLOC * c:RLOC * (c + 1)] = blk

    final_v, sv, ss, swv = full[:, :R]
    dw = np.exp(-np.arange(T, dtype=np.float32) / np.float32(10.0))
    liquid = np.concatenate([
        final_v * np.float32(0.4),
        (sv / np.float32(T)) * np.float32(0.3),
        (ss / np.float32(T)) * np.float32(0.2),
        (swv / dw.sum().astype(np.float32)) * np.float32(0.1),
    ], axis=0).astype(np.float32)  # [8000, 32]
    out = (W_ro @ liquid).T + b_ro
    return out.astype(np.float32)

